# revision 3
# baseline (speedup 1.0000x reference)
"""Trainium2 Bass kernel for ExpertsChooseMaskedExpand MoE routing.

Math (reference):
    xd[b,e,c,i] = sum_t x[b,t,(e,i)] * dmask[b,t,e,c]            (dispatch)
    y[b,e,c,o]  = sum_i xd[b,e,c,i] * w[e,o,i] + bias[o]         (expert mm)
    out[b,t,o]  = sum_{e,c} y[b,e,c,o] * cmb[b,t,e,c]            (combine)

Restructured (combine applied before the weight matmul — 155 GF total
instead of 215 GF; the E expert matmuls fuse into one K=2048 matmul):
    xd[b,e][c,j] = sum_t dmask[b,e][t,c] * xr[b,e][t,j]
    zT[b,e][j,t] = sum_c xd[b,e][c,j] * cmbT[b,e][c,t]
    out[b][t,o]  = sum_{(e,j)} zT[b][(e,j),t] * wstack[(e,j),o] + s[b][t]*bias[o]
    where s[b][t] = sum_{e,c} cmb[b,t,e,c],  wstack[(e,j),o] = w[e,o,j]

Sharding: 8 cores = (batch b in 0..3) x (output half oh in 0..1). Each
core computes its exact out[b][:, oh*4096:(oh+1)*4096] slice - no
cross-core reduction. All matmuls run as float32r (fp22, full PE rate).
The s[t]*bias[o] rank-1 term is fused into the PSUM->SBUF eviction via
scalar_tensor_tensor on the vector engine.
"""

import numpy as np

B, T, E, C = 4, 1024, 4, 512
IN, OUT = 2048, 8192
P = 128
TT = T // P          # 8  t-tiles
CT = C // P          # 4  c-tiles per expert
JT = 4               # j-tiles per expert (i = 512)
KT = E * JT          # 16 k-tiles for the fused matmul (K = 2048)
OH = OUT // 2        # 4096 output columns per core
OC = OH // 512       # 8  output chunks of 512
WQ = 4               # w quarters per oc (4 kt each)

_CACHE = {}


def _build_nc():
    import concourse.mybir as mybir
    import concourse.tile as tile
    from concourse import bacc

    f32 = mybir.dt.float32
    f32r = mybir.dt.float32r

    nc = bacc.Bacc("TRN2", target_bir_lowering=False, debug=False, num_devices=8)
    x_t = nc.dram_tensor("x", (T, IN), f32r, kind="ExternalInput")
    dm_t = nc.dram_tensor("dm", (T, E, C), f32r, kind="ExternalInput")
    cT_t = nc.dram_tensor("cmbT", (E, C, T), f32r, kind="ExternalInput")
    wT_t = nc.dram_tensor("wT", (KT, P, OH), f32r, kind="ExternalInput")
    sT_t = nc.dram_tensor("sT", (P, TT), f32, kind="ExternalInput")
    bb_t = nc.dram_tensor("biasb", (P, OH), f32, kind="ExternalInput")
    o_t = nc.dram_tensor("out", (T, OH), f32, kind="ExternalOutput")

    x_r = x_t.ap().rearrange("(tt p) f -> p tt f", p=P)        # [128, 8, 2048]
    dm_r = dm_t.ap().rearrange("(tt p) e c -> p tt e c", p=P)  # [128, 8, 4, 512]
    cT_r = cT_t.ap().rearrange("e (ct p) t -> p e ct t", p=P)  # [128, 4, 4, 1024]
    wT_r = wT_t.ap().rearrange("kt p o -> p kt o")             # [128, 16, 4096]
    o_r = o_t.ap().rearrange("(tt p) o -> p tt o", p=P)        # [128, 8, 4096]

    with tile.TileContext(nc) as tc:
        with (
            tc.tile_pool(name="persist", bufs=1) as persist,
            tc.tile_pool(name="wp", bufs=6) as wp,
            tc.tile_pool(name="exp", bufs=2) as exp,
            tc.tile_pool(name="ring", bufs=3) as ring,
            tc.tile_pool(name="op", bufs=3) as op,
            tc.tile_pool(name="ps_a", bufs=4, space="PSUM") as ps_a,
            tc.tile_pool(name="ps_b", bufs=2, space="PSUM") as ps_b,
        ):
            zT = persist.tile([P, KT, T], f32r)       # 64 KiB/partition
            sT_sb = persist.tile([P, TT], f32)
            bb_sb = persist.tile([P, OH], f32)        # 16 KiB/partition
            nc.sync.dma_start(sT_sb, sT_t.ap())
            nc.gpsimd.dma_start(bb_sb, bb_t.ap())

            # early weight prefetch (phase 3 inputs) on the gpsimd queue
            w_tiles = {}

            def load_w(oc, q):
                t = wp.tile([P, WQ, 512], f32r, tag="w")
                nc.gpsimd.dma_start(
                    t, wT_r[:, 4 * q : 4 * q + 4, oc * 512 : (oc + 1) * 512]
                )
                w_tiles[(oc, q)] = t

            for oc, q in [(0, 0), (0, 1), (0, 2), (0, 3), (1, 0), (1, 1)]:
                load_w(oc, q)

            # ---- Phases 1+2: per-expert dispatch and combine ----
            for e in range(E):
                # phase 1: xd[c, j] = sum_t dm[t, c] * x[t, j]
                # tt-outer with chunked loads so matmuls start on first chunk
                ps1 = [
                    ps_a.tile([P, 512], f32, tag="ps1", name=f"ps1_{e}_{ct}")
                    for ct in range(CT)
                ]
                for tt in range(TT):
                    x_ch = ring.tile([P, 512], f32r, tag="x")
                    dm_ch = ring.tile([P, 512], f32r, tag="dm")
                    nc.sync.dma_start(x_ch, x_r[:, tt, e * 512 : (e + 1) * 512])
                    nc.sync.dma_start(dm_ch, dm_r[:, tt, e, :])
                    for ct in range(CT):
                        nc.tensor.matmul(
                            ps1[ct],
                            dm_ch[:, ct * P : (ct + 1) * P],
                            x_ch,
                            start=(tt == 0),
                            stop=(tt == TT - 1),
                        )
                xd_e = exp.tile([P, CT, 512], f32r, tag="xd")
                for ct in range(CT):
                    nc.vector.tensor_copy(xd_e[:, ct, :], ps1[ct])

                # phase 2: zT[j, t] = sum_c xd[c, j] * cmbT[c, t]
                for th in range(2):
                    c_th = exp.tile([P, CT, 512], f32r, tag="c")
                    nc.sync.dma_start(
                        c_th, cT_r[:, e, :, th * 512 : (th + 1) * 512]
                    )
                    for jt in range(JT):
                        ps2 = ps_b.tile([P, 512], f32, tag="ps2")
                        for ct in range(CT):
                            nc.tensor.matmul(
                                ps2,
                                xd_e[:, ct, jt * P : (jt + 1) * P],
                                c_th[:, ct, :],
                                start=(ct == 0),
                                stop=(ct == CT - 1),
                            )
                        nc.vector.tensor_copy(
                            zT[:, e * JT + jt, th * 512 : (th + 1) * 512], ps2
                        )

            # ---- Phase 3: out[t,o] = sum_kt zT[kt].T @ w[kt] + s*bias ----
            for oc in range(OC):
                for q in range(WQ):
                    if (oc, q) not in w_tiles:
                        load_w(oc, q)
                for tt in range(TT):
                    ps3 = ps_b.tile([P, 512], f32, tag="ps3")
                    for kt in range(KT):
                        nc.tensor.matmul(
                            ps3,
                            zT[:, kt, tt * P : (tt + 1) * P],
                            w_tiles[(oc, kt // WQ)][:, kt % WQ, :],
                            start=(kt == 0),
                            stop=(kt == KT - 1),
                        )
                    o_sb = op.tile([P, 512], f32, tag="o_sb")
                    # out = biasb[:, oc] * sT[:, tt] + psum
                    nc.vector.scalar_tensor_tensor(
                        o_sb,
                        bb_sb[:, oc * 512 : (oc + 1) * 512],
                        sT_sb[:, tt : tt + 1],
                        ps3,
                        mybir.AluOpType.mult,
                        mybir.AluOpType.add,
                    )
                    nc.gpsimd.dma_start(o_r[:, tt, oc * 512 : (oc + 1) * 512], o_sb)
                # prefetch weights for later oc groups
                if oc + 1 < OC:
                    for q, (poc, pq) in enumerate(
                        [(oc + 1, 2), (oc + 1, 3), (oc + 2, 0), (oc + 2, 1)]
                    ):
                        if poc < OC and (poc, pq) not in w_tiles:
                            load_w(poc, pq)

    nc.compile()
    return nc


def _get_nc():
    if "nc" not in _CACHE:
        _CACHE["nc"] = _build_nc()
    return _CACHE["nc"]


def _prep_in_maps(x, combine_array, dispatch_mask, weight, bias):
    x = np.ascontiguousarray(x, dtype=np.float32)
    cmb = np.ascontiguousarray(combine_array, dtype=np.float32)
    dm = np.ascontiguousarray(dispatch_mask, dtype=np.float32)
    weight = np.ascontiguousarray(weight, dtype=np.float32)
    bias = np.ascontiguousarray(bias, dtype=np.float32)

    # combine transposed to (B, E, C, T) so that C lands on partitions
    cmbT = np.ascontiguousarray(cmb.transpose(0, 2, 3, 1))
    s = cmb.sum(axis=(2, 3))  # (B, T)
    sT = [np.ascontiguousarray(s[b].reshape(TT, P).T) for b in range(B)]  # (P, TT)
    # wstack[(e,j), o] = w[e, o, j];  w = weight.reshape(E, OUT, IN//E)
    w = weight.reshape(E, OUT, IN // E)
    wstack = np.ascontiguousarray(w.transpose(0, 2, 1)).reshape(IN, OUT)
    wT = [
        np.ascontiguousarray(wstack[:, oh * OH : (oh + 1) * OH]).reshape(KT, P, OH)
        for oh in range(2)
    ]
    biasb = [
        np.ascontiguousarray(
            np.broadcast_to(bias[oh * OH : (oh + 1) * OH], (P, OH))
        )
        for oh in range(2)
    ]

    in_maps = []
    for k in range(8):
        b, oh = k // 2, k % 2
        in_maps.append(
            {
                "x": x[b],
                "dm": dm[b],
                "cmbT": cmbT[b],
                "wT": wT[oh],
                "sT": sT[b],
                "biasb": biasb[oh],
            }
        )
    return in_maps


def run_spmd(in_maps, trace=False, **kwargs):
    from concourse.bass_utils import run_bass_kernel_spmd

    nc = _get_nc()
    return run_bass_kernel_spmd(
        nc, in_maps, core_ids=list(range(8)), trace=trace, **kwargs
    )


def kernel(x, combine_array, dispatch_mask, weight, bias, num_experts):
    assert int(num_experts) == E
    in_maps = _prep_in_maps(x, combine_array, dispatch_mask, weight, bias)
    res = run_spmd(in_maps)
    out = np.empty((B, T, OUT), dtype=np.float32)
    for k in range(8):
        b, oh = k // 2, k % 2
        out[b, :, oh * OH : (oh + 1) * OH] = res.results[k]["out"]
    return out


# revision 5
# speedup vs baseline: 1.1494x; 1.1494x over previous
"""Trainium2 Bass kernel for ExpertsChooseMaskedExpand MoE routing.

Math (reference):
    xd[b,e,c,i] = sum_t x[b,t,(e,i)] * dmask[b,t,e,c]            (dispatch)
    y[b,e,c,o]  = sum_i xd[b,e,c,i] * w[e,o,i] + bias[o]         (expert mm)
    out[b,t,o]  = sum_{e,c} y[b,e,c,o] * cmb[b,t,e,c]            (combine)

Restructured (combine applied before the weight matmul — 155 GF total
instead of 215 GF; the E expert matmuls fuse into one K=2048 matmul):
    xd[b,e][c,j] = sum_t dmask[b,e][t,c] * xr[b,e][t,j]
    zT[b,e][j,t] = sum_c xd[b,e][c,j] * cmbT[b,e][c,t]
    out[b][t,o]  = sum_{(e,j)} zT[b][(e,j),t] * wstack[(e,j),o] + s[b][t]*bias[o]
    where s[b][t] = sum_{e,c} cmb[b,t,e,c],  wstack[(e,j),o] = w[e,o,j]

Sharding: 8 cores = (batch b in 0..3) x (output half oh in 0..1). Each
core computes its exact out[b][:, oh*4096:(oh+1)*4096] slice - no
cross-core reduction. All matmuls run as float32r (fp22, full PE rate).
The s[t]*bias[o] rank-1 term is a K=1 matmul appended to each
accumulation group.
"""

import numpy as np

B, T, E, C = 4, 1024, 4, 512
IN, OUT = 2048, 8192
P = 128
TT = T // P          # 8  t-tiles
CT = C // P          # 4  c-tiles per expert
JT = 4               # j-tiles per expert (i = 512)
KT = E * JT          # 16 k-tiles for the fused matmul (K = 2048)
OH = OUT // 2        # 4096 output columns per core
OC = OH // 512       # 8  output chunks of 512
WQ = 4               # w quarters per oc (4 kt each)

_CACHE = {}


def _build_nc():
    import concourse.mybir as mybir
    import concourse.tile as tile
    from concourse import bacc

    f32 = mybir.dt.float32
    f32r = mybir.dt.float32r

    nc = bacc.Bacc("TRN2", target_bir_lowering=False, debug=False, num_devices=8)
    x_t = nc.dram_tensor("x", (T, IN), f32r, kind="ExternalInput")
    dm_t = nc.dram_tensor("dm", (T, E, C), f32r, kind="ExternalInput")
    cT_t = nc.dram_tensor("cmbT", (E, C, T), f32r, kind="ExternalInput")
    wT_t = nc.dram_tensor("wT", (KT, P, OH), f32r, kind="ExternalInput")
    s_t = nc.dram_tensor("s", (1, T), f32r, kind="ExternalInput")
    b_t = nc.dram_tensor("bias", (1, OH), f32r, kind="ExternalInput")
    o_t = nc.dram_tensor("out", (T, OH), f32, kind="ExternalOutput")

    x_r = x_t.ap().rearrange("(tt p) f -> p tt f", p=P)        # [128, 8, 2048]
    dm_r = dm_t.ap().rearrange("(tt p) e c -> p tt e c", p=P)  # [128, 8, 4, 512]
    cT_r = cT_t.ap().rearrange("e (ct p) t -> p e ct t", p=P)  # [128, 4, 4, 1024]
    wT_r = wT_t.ap().rearrange("kt p o -> p kt o")             # [128, 16, 4096]
    o_r = o_t.ap().rearrange("(tt p) o -> p tt o", p=P)        # [128, 8, 4096]

    with tile.TileContext(nc) as tc:
        with (
            tc.tile_pool(name="persist", bufs=1) as persist,
            tc.tile_pool(name="wp", bufs=6) as wp,
            tc.tile_pool(name="exp", bufs=2) as exp,
            tc.tile_pool(name="op", bufs=2) as op,
        ):
            zT = persist.tile([P, KT, T], f32r)       # 64 KiB/partition
            s_sb = persist.tile([1, T], f32r)
            bias_sb = persist.tile([1, OH], f32r)
            nc.sync.dma_start(s_sb, s_t.ap())
            nc.sync.dma_start(bias_sb, b_t.ap())

            w_tiles = {}

            def load_w(oc, q):
                t = wp.tile([P, WQ, 512], f32r, tag="w", name=f"w_{oc}_{q}")
                nc.sync.dma_start(
                    t, wT_r[:, WQ * q : WQ * (q + 1), oc * 512 : (oc + 1) * 512]
                )
                w_tiles[(oc, q)] = t

            # ---- Phases 1+2: per-expert dispatch and combine ----
            with (
                tc.tile_pool(name="ps_a", bufs=4, space="PSUM") as ps_a,
                tc.tile_pool(name="ps_b", bufs=2, space="PSUM") as ps_b,
            ):
                for e in range(E):
                    # phase 1: xd[c, j] = sum_t dm[t, c] * x[t, j]
                    # loads split in halves; tt-outer so matmuls start early
                    xh, dmh = {}, {}
                    for h in range(2):
                        hs = slice(h * 4, h * 4 + 4)
                        xh[h] = exp.tile([P, 4, 512], f32r, tag="x", name=f"x_{e}_{h}")
                        dmh[h] = exp.tile(
                            [P, 4, 512], f32r, tag="dm", name=f"dm_{e}_{h}"
                        )
                        nc.sync.dma_start(
                            xh[h], x_r[:, hs, e * 512 : (e + 1) * 512]
                        )
                        nc.sync.dma_start(dmh[h], dm_r[:, hs, e, :])
                    ps1 = [
                        ps_a.tile([P, 512], f32, tag="ps1", name=f"ps1_{e}_{ct}")
                        for ct in range(CT)
                    ]
                    for tt in range(TT):
                        h, hi = tt // 4, tt % 4
                        for ct in range(CT):
                            nc.tensor.matmul(
                                ps1[ct],
                                dmh[h][:, hi, ct * P : (ct + 1) * P],
                                xh[h][:, hi, :],
                                start=(tt == 0),
                                stop=(tt == TT - 1),
                            )
                    xd_e = exp.tile([P, CT, 512], f32r, tag="xd")
                    for ct in range(CT):
                        nc.vector.tensor_copy(xd_e[:, ct, :], ps1[ct])

                    # interleave weight prefetch into the sync DMA stream
                    for oc, q in [(0, 2 * e), (0, 2 * e + 1)] if e < 2 else [
                        (1, 2 * (e - 2)),
                        (1, 2 * (e - 2) + 1),
                    ]:
                        load_w(oc, q)

                    # phase 2: zT[j, t] = sum_c xd[c, j] * cmbT[c, t]
                    for th in range(2):
                        c_th = exp.tile([P, CT, 512], f32r, tag="c")
                        nc.sync.dma_start(
                            c_th, cT_r[:, e, :, th * 512 : (th + 1) * 512]
                        )
                        for jt in range(JT):
                            ps2 = ps_b.tile([P, 512], f32, tag="ps2")
                            for ct in range(CT):
                                nc.tensor.matmul(
                                    ps2,
                                    xd_e[:, ct, jt * P : (jt + 1) * P],
                                    c_th[:, ct, :],
                                    start=(ct == 0),
                                    stop=(ct == CT - 1),
                                )
                            nc.vector.tensor_copy(
                                zT[:, e * JT + jt, th * 512 : (th + 1) * 512], ps2
                            )

            # ---- Phase 3: out[t,o] = sum_kt zT[kt].T @ w[kt] + s^T bias ----
            with tc.tile_pool(name="ps_c", bufs=4, space="PSUM") as ps_c:
                for oc in range(OC):
                    for q in range(WQ):
                        if (oc, q) not in w_tiles:
                            load_w(oc, q)
                    for tt in range(TT):
                        ps3 = ps_c.tile([P, 512], f32, tag="ps3")
                        for kt in range(KT):
                            nc.tensor.matmul(
                                ps3,
                                zT[:, kt, tt * P : (tt + 1) * P],
                                w_tiles[(oc, kt // WQ)][:, kt % WQ, :],
                                start=(kt == 0),
                                stop=False,
                            )
                        # rank-1 bias update: += s[t] * bias[o]
                        nc.tensor.matmul(
                            ps3,
                            s_sb[:, tt * P : (tt + 1) * P],
                            bias_sb[:, oc * 512 : (oc + 1) * 512],
                            start=False,
                            stop=True,
                        )
                        o_sb = op.tile([P, 512], f32, tag="o_sb")
                        nc.vector.tensor_copy(o_sb, ps3)
                        nc.sync.dma_start(
                            o_r[:, tt, oc * 512 : (oc + 1) * 512], o_sb
                        )
                    # prefetch weights for later oc groups
                    for poc, pq in [(oc + 1, 2), (oc + 1, 3), (oc + 2, 0), (oc + 2, 1)]:
                        if poc < OC and (poc, pq) not in w_tiles:
                            load_w(poc, pq)

    nc.compile()
    return nc


def _get_nc():
    if "nc" not in _CACHE:
        _CACHE["nc"] = _build_nc()
    return _CACHE["nc"]


def _prep_in_maps(x, combine_array, dispatch_mask, weight, bias):
    x = np.ascontiguousarray(x, dtype=np.float32)
    cmb = np.ascontiguousarray(combine_array, dtype=np.float32)
    dm = np.ascontiguousarray(dispatch_mask, dtype=np.float32)
    weight = np.ascontiguousarray(weight, dtype=np.float32)
    bias = np.ascontiguousarray(bias, dtype=np.float32)

    # combine transposed to (B, E, C, T) so that C lands on partitions
    cmbT = np.ascontiguousarray(cmb.transpose(0, 2, 3, 1))
    s = cmb.sum(axis=(2, 3))  # (B, T)
    # wstack[(e,j), o] = w[e, o, j];  w = weight.reshape(E, OUT, IN//E)
    w = weight.reshape(E, OUT, IN // E)
    wstack = np.ascontiguousarray(w.transpose(0, 2, 1)).reshape(IN, OUT)
    wT = [
        np.ascontiguousarray(wstack[:, oh * OH : (oh + 1) * OH]).reshape(KT, P, OH)
        for oh in range(2)
    ]
    bias_h = [np.ascontiguousarray(bias[oh * OH : (oh + 1) * OH]) for oh in range(2)]

    in_maps = []
    for k in range(8):
        b, oh = k // 2, k % 2
        in_maps.append(
            {
                "x": x[b],
                "dm": dm[b],
                "cmbT": cmbT[b],
                "wT": wT[oh],
                "s": s[b : b + 1],
                "bias": bias_h[oh].reshape(1, OH),
            }
        )
    return in_maps


def run_spmd(in_maps, trace=False, **kwargs):
    from concourse.bass_utils import run_bass_kernel_spmd

    nc = _get_nc()
    return run_bass_kernel_spmd(
        nc, in_maps, core_ids=list(range(8)), trace=trace, **kwargs
    )


def kernel(x, combine_array, dispatch_mask, weight, bias, num_experts):
    assert int(num_experts) == E
    in_maps = _prep_in_maps(x, combine_array, dispatch_mask, weight, bias)
    res = run_spmd(in_maps)
    out = np.empty((B, T, OUT), dtype=np.float32)
    for k in range(8):
        b, oh = k // 2, k % 2
        out[b, :, oh * OH : (oh + 1) * OH] = res.results[k]["out"]
    return out


# revision 6
# speedup vs baseline: 1.2751x; 1.1093x over previous
"""Trainium2 Bass kernel for ExpertsChooseMaskedExpand MoE routing.

Math (reference):
    xd[b,e,c,i] = sum_t x[b,t,(e,i)] * dmask[b,t,e,c]            (dispatch)
    y[b,e,c,o]  = sum_i xd[b,e,c,i] * w[e,o,i] + bias[o]         (expert mm)
    out[b,t,o]  = sum_{e,c} y[b,e,c,o] * cmb[b,t,e,c]            (combine)

Restructured (combine applied before the weight matmul — 155 GF total
instead of 215 GF; the E expert matmuls fuse into one K=2048 matmul):
    xd[b,e][c,j] = sum_t dmask[b,e][t,c] * xr[b,e][t,j]
    zT[b,e][j,t] = sum_c xd[b,e][c,j] * cmbT[b,e][c,t]
    out[b][t,o]  = sum_{(e,j)} zT[b][(e,j),t] * wstack[(e,j),o] + s[b][t]*bias[o]
    where s[b][t] = sum_{e,c} cmb[b,t,e,c],  wstack[(e,j),o] = w[e,o,j]

Sharding: 8 cores = (batch b in 0..3) x (output half oh in 0..1). Each
core computes its exact out[b][:, oh*4096:(oh+1)*4096] slice - no
cross-core reduction. All matmuls run as float32r (fp22, full PE rate).
The s[t]*bias[o] rank-1 term is a K=1 matmul appended to each
accumulation group.
"""

import numpy as np

B, T, E, C = 4, 1024, 4, 512
IN, OUT = 2048, 8192
P = 128
TT = T // P          # 8  t-tiles
CT = C // P          # 4  c-tiles per expert
JT = 4               # j-tiles per expert (i = 512)
KT = E * JT          # 16 k-tiles for the fused matmul (K = 2048)
OH = OUT // 2        # 4096 output columns per core
OC = OH // 512       # 8  output chunks of 512
WQ = 4               # w quarters per oc (4 kt each)

_CACHE = {}


def _build_nc():
    import concourse.mybir as mybir
    import concourse.tile as tile
    from concourse import bacc

    f32 = mybir.dt.float32
    f32r = mybir.dt.float32r

    nc = bacc.Bacc("TRN2", target_bir_lowering=False, debug=False, num_devices=8)
    x_t = nc.dram_tensor("x", (T, IN), f32r, kind="ExternalInput")
    dm_t = nc.dram_tensor("dm", (T, E, C), f32r, kind="ExternalInput")
    cT_t = nc.dram_tensor("cmbT", (E, C, T), f32r, kind="ExternalInput")
    wT_t = nc.dram_tensor("wT", (KT, P, OH), f32r, kind="ExternalInput")
    sT_t = nc.dram_tensor("sT", (P, TT), f32, kind="ExternalInput")
    bb_t = nc.dram_tensor("biasb", (P, OH), f32, kind="ExternalInput")
    o_t = nc.dram_tensor("out", (T, OH), f32, kind="ExternalOutput")

    x_r = x_t.ap().rearrange("(tt p) f -> p tt f", p=P)        # [128, 8, 2048]
    dm_r = dm_t.ap().rearrange("(tt p) e c -> p tt e c", p=P)  # [128, 8, 4, 512]
    cT_r = cT_t.ap().rearrange("e (ct p) t -> p e ct t", p=P)  # [128, 4, 4, 1024]
    wT_r = wT_t.ap().rearrange("kt p o -> p kt o")             # [128, 16, 4096]
    o_r = o_t.ap().rearrange("(tt p) o -> p tt o", p=P)        # [128, 8, 4096]

    with tile.TileContext(nc) as tc:
        with (
            tc.tile_pool(name="persist", bufs=1) as persist,
            tc.tile_pool(name="wp", bufs=6) as wp,
            tc.tile_pool(name="exp", bufs=2) as exp,
            tc.tile_pool(name="op", bufs=2) as op,
        ):
            zT = persist.tile([P, KT, T], f32r)       # 64 KiB/partition
            sT_sb = persist.tile([P, TT], f32)
            nc.sync.dma_start(sT_sb, sT_t.ap())

            w_tiles = {}

            def load_w(oc, q):
                t = wp.tile([P, WQ, 512], f32r, tag="w", name=f"w_{oc}_{q}")
                nc.sync.dma_start(
                    t, wT_r[:, WQ * q : WQ * (q + 1), oc * 512 : (oc + 1) * 512]
                )
                w_tiles[(oc, q)] = t

            # ---- Phases 1+2: per-expert dispatch and combine ----
            with (
                tc.tile_pool(name="ps_a", bufs=4, space="PSUM") as ps_a,
                tc.tile_pool(name="ps_b", bufs=2, space="PSUM") as ps_b,
            ):
                for e in range(E):
                    # phase 1: xd[c, j] = sum_t dm[t, c] * x[t, j]
                    # loads split in halves; tt-outer so matmuls start early
                    xh, dmh = {}, {}
                    for h in range(2):
                        hs = slice(h * 4, h * 4 + 4)
                        xh[h] = exp.tile([P, 4, 512], f32r, tag="x", name=f"x_{e}_{h}")
                        dmh[h] = exp.tile(
                            [P, 4, 512], f32r, tag="dm", name=f"dm_{e}_{h}"
                        )
                        nc.sync.dma_start(
                            xh[h], x_r[:, hs, e * 512 : (e + 1) * 512]
                        )
                        nc.sync.dma_start(dmh[h], dm_r[:, hs, e, :])
                    ps1 = [
                        ps_a.tile([P, 512], f32, tag="ps1", name=f"ps1_{e}_{ct}")
                        for ct in range(CT)
                    ]
                    for tt in range(TT):
                        h, hi = tt // 4, tt % 4
                        for ct in range(CT):
                            nc.tensor.matmul(
                                ps1[ct],
                                dmh[h][:, hi, ct * P : (ct + 1) * P],
                                xh[h][:, hi, :],
                                start=(tt == 0),
                                stop=(tt == TT - 1),
                            )
                    xd_e = exp.tile([P, CT, 512], f32r, tag="xd")
                    for ct in range(CT):
                        nc.vector.tensor_copy(xd_e[:, ct, :], ps1[ct])

                    # phase 2: zT[j, t] = sum_c xd[c, j] * cmbT[c, t]
                    for th in range(2):
                        c_th = exp.tile([P, CT, 512], f32r, tag="c")
                        nc.sync.dma_start(
                            c_th, cT_r[:, e, :, th * 512 : (th + 1) * 512]
                        )
                        for jt in range(JT):
                            ps2 = ps_b.tile([P, 512], f32, tag="ps2")
                            for ct in range(CT):
                                nc.tensor.matmul(
                                    ps2,
                                    xd_e[:, ct, jt * P : (jt + 1) * P],
                                    c_th[:, ct, :],
                                    start=(ct == 0),
                                    stop=(ct == CT - 1),
                                )
                            nc.vector.tensor_copy(
                                zT[:, e * JT + jt, th * 512 : (th + 1) * 512], ps2
                            )

                    # weight prefetch at the tail of each expert's DMA stream:
                    # oc0 quarters during experts 1-2, oc1 q0/q1 after expert 3
                    tail_w = {
                        1: [(0, 0), (0, 1)],
                        2: [(0, 2), (0, 3)],
                        3: [(1, 0), (1, 1)],
                    }.get(e, [])
                    for oc, q in tail_w:
                        load_w(oc, q)

            # ---- Phase 3: out[t,o] = sum_kt zT[kt].T @ w[kt] + s^T bias ----
            with tc.tile_pool(name="ps_c", bufs=4, space="PSUM") as ps_c:
                for oc in range(OC):
                    for q in range(WQ):
                        if (oc, q) not in w_tiles:
                            load_w(oc, q)
                    bias_oc = op.tile([P, 512], f32, tag="bias_oc")
                    nc.sync.dma_start(
                        bias_oc, bb_t.ap()[:, oc * 512 : (oc + 1) * 512]
                    )
                    for tt in range(TT):
                        ps3 = ps_c.tile([P, 512], f32, tag="ps3")
                        for kt in range(KT):
                            nc.tensor.matmul(
                                ps3,
                                zT[:, kt, tt * P : (tt + 1) * P],
                                w_tiles[(oc, kt // WQ)][:, kt % WQ, :],
                                start=(kt == 0),
                                stop=(kt == KT - 1),
                            )
                        o_sb = op.tile([P, 512], f32, tag="o_sb")
                        # out = biasb[:, oc] * sT[:, tt] + psum
                        nc.vector.scalar_tensor_tensor(
                            o_sb,
                            bias_oc,
                            sT_sb[:, tt : tt + 1],
                            ps3,
                            mybir.AluOpType.mult,
                            mybir.AluOpType.add,
                        )
                        nc.sync.dma_start(
                            o_r[:, tt, oc * 512 : (oc + 1) * 512], o_sb
                        )
                    # prefetch weights for later oc groups
                    for poc, pq in [(oc + 1, 2), (oc + 1, 3), (oc + 2, 0), (oc + 2, 1)]:
                        if poc < OC and (poc, pq) not in w_tiles:
                            load_w(poc, pq)

    nc.compile()
    return nc


def _get_nc():
    if "nc" not in _CACHE:
        _CACHE["nc"] = _build_nc()
    return _CACHE["nc"]


def _prep_in_maps(x, combine_array, dispatch_mask, weight, bias):
    x = np.ascontiguousarray(x, dtype=np.float32)
    cmb = np.ascontiguousarray(combine_array, dtype=np.float32)
    dm = np.ascontiguousarray(dispatch_mask, dtype=np.float32)
    weight = np.ascontiguousarray(weight, dtype=np.float32)
    bias = np.ascontiguousarray(bias, dtype=np.float32)

    # combine transposed to (B, E, C, T) so that C lands on partitions
    cmbT = np.ascontiguousarray(cmb.transpose(0, 2, 3, 1))
    s = cmb.sum(axis=(2, 3))  # (B, T)
    sT = [np.ascontiguousarray(s[b].reshape(TT, P).T) for b in range(B)]  # (P, TT)
    # wstack[(e,j), o] = w[e, o, j];  w = weight.reshape(E, OUT, IN//E)
    w = weight.reshape(E, OUT, IN // E)
    wstack = np.ascontiguousarray(w.transpose(0, 2, 1)).reshape(IN, OUT)
    wT = [
        np.ascontiguousarray(wstack[:, oh * OH : (oh + 1) * OH]).reshape(KT, P, OH)
        for oh in range(2)
    ]
    biasb = [
        np.ascontiguousarray(np.broadcast_to(bias[oh * OH : (oh + 1) * OH], (P, OH)))
        for oh in range(2)
    ]

    in_maps = []
    for k in range(8):
        b, oh = k // 2, k % 2
        in_maps.append(
            {
                "x": x[b],
                "dm": dm[b],
                "cmbT": cmbT[b],
                "wT": wT[oh],
                "sT": sT[b],
                "biasb": biasb[oh],
            }
        )
    return in_maps


def run_spmd(in_maps, trace=False, **kwargs):
    from concourse.bass_utils import run_bass_kernel_spmd

    nc = _get_nc()
    return run_bass_kernel_spmd(
        nc, in_maps, core_ids=list(range(8)), trace=trace, **kwargs
    )


def kernel(x, combine_array, dispatch_mask, weight, bias, num_experts):
    assert int(num_experts) == E
    in_maps = _prep_in_maps(x, combine_array, dispatch_mask, weight, bias)
    res = run_spmd(in_maps)
    out = np.empty((B, T, OUT), dtype=np.float32)
    for k in range(8):
        b, oh = k // 2, k % 2
        out[b, :, oh * OH : (oh + 1) * OH] = res.results[k]["out"]
    return out


# revision 7
# speedup vs baseline: 1.3259x; 1.0399x over previous
"""Trainium2 Bass kernel for ExpertsChooseMaskedExpand MoE routing.

Math (reference):
    xd[b,e,c,i] = sum_t x[b,t,(e,i)] * dmask[b,t,e,c]            (dispatch)
    y[b,e,c,o]  = sum_i xd[b,e,c,i] * w[e,o,i] + bias[o]         (expert mm)
    out[b,t,o]  = sum_{e,c} y[b,e,c,o] * cmb[b,t,e,c]            (combine)

Restructured (combine applied before the weight matmul — 155 GF total
instead of 215 GF; the E expert matmuls fuse into one K=2048 matmul):
    xd[b,e][c,j] = sum_t dmask[b,e][t,c] * xr[b,e][t,j]
    zT[b,e][j,t] = sum_c xd[b,e][c,j] * cmbT[b,e][c,t]
    out[b][t,o]  = sum_{(e,j)} zT[b][(e,j),t] * wstack[(e,j),o] + s[b][t]*bias[o]
    where s[b][t] = sum_{e,c} cmb[b,t,e,c],  wstack[(e,j),o] = w[e,o,j]

Sharding: 8 cores = (batch b in 0..3) x (output half oh in 0..1). Each
core computes its exact out[b][:, oh*4096:(oh+1)*4096] slice - no
cross-core reduction. All matmuls run as float32r (fp22, full PE rate).
The s[t]*bias[o] rank-1 term is a K=1 matmul appended to each
accumulation group.
"""

import numpy as np

B, T, E, C = 4, 1024, 4, 512
IN, OUT = 2048, 8192
P = 128
TT = T // P          # 8  t-tiles
CT = C // P          # 4  c-tiles per expert
JT = 4               # j-tiles per expert (i = 512)
KT = E * JT          # 16 k-tiles for the fused matmul (K = 2048)
OH = OUT // 2        # 4096 output columns per core
OC = OH // 512       # 8  output chunks of 512
WQ = 4               # w quarters per oc (4 kt each)

_CACHE = {}


def _build_nc():
    import concourse.mybir as mybir
    import concourse.tile as tile
    from concourse import bacc

    f32 = mybir.dt.float32
    f32r = mybir.dt.float32r

    nc = bacc.Bacc("TRN2", target_bir_lowering=False, debug=False, num_devices=8)
    x_t = nc.dram_tensor("x", (T, IN), f32r, kind="ExternalInput")
    dm_t = nc.dram_tensor("dm", (T, E, C), f32r, kind="ExternalInput")
    cT_t = nc.dram_tensor("cmbT", (E, C, T), f32r, kind="ExternalInput")
    wT_t = nc.dram_tensor("wT", (KT, P, OH), f32r, kind="ExternalInput")
    sT_t = nc.dram_tensor("sT", (P, TT), f32, kind="ExternalInput")
    bb_t = nc.dram_tensor("biasb", (P, OH), f32, kind="ExternalInput")
    o_t = nc.dram_tensor("out", (T, OH), f32, kind="ExternalOutput")

    x_r = x_t.ap().rearrange("(tt p) f -> p tt f", p=P)        # [128, 8, 2048]
    dm_r = dm_t.ap().rearrange("(tt p) e c -> p tt e c", p=P)  # [128, 8, 4, 512]
    cT_r = cT_t.ap().rearrange("e (ct p) t -> p e ct t", p=P)  # [128, 4, 4, 1024]
    wT_r = wT_t.ap().rearrange("kt p o -> p kt o")             # [128, 16, 4096]
    o_r = o_t.ap().rearrange("(tt p) o -> p tt o", p=P)        # [128, 8, 4096]

    with tile.TileContext(nc) as tc:
        with (
            tc.tile_pool(name="persist", bufs=1) as persist,
            tc.tile_pool(name="wp", bufs=8) as wp,
            tc.tile_pool(name="exp", bufs=2) as exp,
            tc.tile_pool(name="xdp", bufs=1) as xdp,
            tc.tile_pool(name="op", bufs=2) as op,
        ):
            zT = persist.tile([P, KT, T], f32r)       # 64 KiB/partition
            sT_sb = persist.tile([P, TT], f32)
            nc.sync.dma_start(sT_sb, sT_t.ap())

            w_tiles = {}

            def load_w(oc, q):
                t = wp.tile([P, WQ, 512], f32r, tag="w", name=f"w_{oc}_{q}")
                nc.sync.dma_start(
                    t, wT_r[:, WQ * q : WQ * (q + 1), oc * 512 : (oc + 1) * 512]
                )
                w_tiles[(oc, q)] = t

            # ---- Phases 1+2: per-expert dispatch and combine ----
            with (
                tc.tile_pool(name="ps_a", bufs=4, space="PSUM") as ps_a,
                tc.tile_pool(name="ps_b", bufs=2, space="PSUM") as ps_b,
            ):
                for e in range(E):
                    # phase 1: xd[c, j] = sum_t dm[t, c] * x[t, j]
                    # loads split in halves; tt-outer so matmuls start early
                    xh, dmh = {}, {}
                    for h in range(2):
                        hs = slice(h * 4, h * 4 + 4)
                        xh[h] = exp.tile([P, 4, 512], f32r, tag="x", name=f"x_{e}_{h}")
                        dmh[h] = exp.tile(
                            [P, 4, 512], f32r, tag="dm", name=f"dm_{e}_{h}"
                        )
                        nc.sync.dma_start(
                            xh[h], x_r[:, hs, e * 512 : (e + 1) * 512]
                        )
                        nc.sync.dma_start(dmh[h], dm_r[:, hs, e, :])
                    ps1 = [
                        ps_a.tile([P, 512], f32, tag="ps1", name=f"ps1_{e}_{ct}")
                        for ct in range(CT)
                    ]
                    for tt in range(TT):
                        h, hi = tt // 4, tt % 4
                        for ct in range(CT):
                            nc.tensor.matmul(
                                ps1[ct],
                                dmh[h][:, hi, ct * P : (ct + 1) * P],
                                xh[h][:, hi, :],
                                start=(tt == 0),
                                stop=(tt == TT - 1),
                            )
                    xd_e = xdp.tile([P, CT, 512], f32r, tag="xd")
                    for ct in range(CT):
                        nc.vector.tensor_copy(xd_e[:, ct, :], ps1[ct])

                    # phase 2: zT[j, t] = sum_c xd[c, j] * cmbT[c, t]
                    for th in range(2):
                        c_th = exp.tile([P, CT, 512], f32r, tag="c")
                        nc.sync.dma_start(
                            c_th, cT_r[:, e, :, th * 512 : (th + 1) * 512]
                        )
                        for jt in range(JT):
                            ps2 = ps_b.tile([P, 512], f32, tag="ps2")
                            for ct in range(CT):
                                nc.tensor.matmul(
                                    ps2,
                                    xd_e[:, ct, jt * P : (jt + 1) * P],
                                    c_th[:, ct, :],
                                    start=(ct == 0),
                                    stop=(ct == CT - 1),
                                )
                            nc.vector.tensor_copy(
                                zT[:, e * JT + jt, th * 512 : (th + 1) * 512], ps2
                            )

                    # weight prefetch at the tail of each expert's DMA stream:
                    # oc0 quarters during experts 1-2, oc1 q0/q1 after expert 3
                    tail_w = {
                        1: [(0, 0), (0, 1)],
                        2: [(0, 2), (0, 3)],
                        3: [(1, 0), (1, 1)],
                    }.get(e, [])
                    for oc, q in tail_w:
                        load_w(oc, q)

            # ---- Phase 3: out[t,o] = sum_kt zT[kt].T @ w[kt] + s^T bias ----
            with tc.tile_pool(name="ps_c", bufs=4, space="PSUM") as ps_c:
                for oc in range(OC):
                    for q in range(WQ):
                        if (oc, q) not in w_tiles:
                            load_w(oc, q)
                    bias_oc = op.tile([P, 512], f32, tag="bias_oc")
                    nc.sync.dma_start(
                        bias_oc, bb_t.ap()[:, oc * 512 : (oc + 1) * 512]
                    )
                    for tt in range(TT):
                        # spread weight prefetch for oc+1 across this oc
                        if tt in (1, 3, 5, 7) and oc + 1 < OC:
                            pq = (tt - 1) // 2
                            if (oc + 1, pq) not in w_tiles:
                                load_w(oc + 1, pq)
                        ps3 = ps_c.tile([P, 512], f32, tag="ps3")
                        for kt in range(KT):
                            nc.tensor.matmul(
                                ps3,
                                zT[:, kt, tt * P : (tt + 1) * P],
                                w_tiles[(oc, kt // WQ)][:, kt % WQ, :],
                                start=(kt == 0),
                                stop=(kt == KT - 1),
                            )
                        o_sb = op.tile([P, 512], f32, tag="o_sb")
                        # out = biasb[:, oc] * sT[:, tt] + psum
                        nc.vector.scalar_tensor_tensor(
                            o_sb,
                            bias_oc,
                            sT_sb[:, tt : tt + 1],
                            ps3,
                            mybir.AluOpType.mult,
                            mybir.AluOpType.add,
                        )
                        nc.sync.dma_start(
                            o_r[:, tt, oc * 512 : (oc + 1) * 512], o_sb
                        )

    nc.compile()
    return nc


def _get_nc():
    if "nc" not in _CACHE:
        _CACHE["nc"] = _build_nc()
    return _CACHE["nc"]


def _prep_in_maps(x, combine_array, dispatch_mask, weight, bias):
    x = np.ascontiguousarray(x, dtype=np.float32)
    cmb = np.ascontiguousarray(combine_array, dtype=np.float32)
    dm = np.ascontiguousarray(dispatch_mask, dtype=np.float32)
    weight = np.ascontiguousarray(weight, dtype=np.float32)
    bias = np.ascontiguousarray(bias, dtype=np.float32)

    # combine transposed to (B, E, C, T) so that C lands on partitions
    cmbT = np.ascontiguousarray(cmb.transpose(0, 2, 3, 1))
    s = cmb.sum(axis=(2, 3))  # (B, T)
    sT = [np.ascontiguousarray(s[b].reshape(TT, P).T) for b in range(B)]  # (P, TT)
    # wstack[(e,j), o] = w[e, o, j];  w = weight.reshape(E, OUT, IN//E)
    w = weight.reshape(E, OUT, IN // E)
    wstack = np.ascontiguousarray(w.transpose(0, 2, 1)).reshape(IN, OUT)
    wT = [
        np.ascontiguousarray(wstack[:, oh * OH : (oh + 1) * OH]).reshape(KT, P, OH)
        for oh in range(2)
    ]
    biasb = [
        np.ascontiguousarray(np.broadcast_to(bias[oh * OH : (oh + 1) * OH], (P, OH)))
        for oh in range(2)
    ]

    in_maps = []
    for k in range(8):
        b, oh = k // 2, k % 2
        in_maps.append(
            {
                "x": x[b],
                "dm": dm[b],
                "cmbT": cmbT[b],
                "wT": wT[oh],
                "sT": sT[b],
                "biasb": biasb[oh],
            }
        )
    return in_maps


def run_spmd(in_maps, trace=False, **kwargs):
    from concourse.bass_utils import run_bass_kernel_spmd

    nc = _get_nc()
    return run_bass_kernel_spmd(
        nc, in_maps, core_ids=list(range(8)), trace=trace, **kwargs
    )


def kernel(x, combine_array, dispatch_mask, weight, bias, num_experts):
    assert int(num_experts) == E
    in_maps = _prep_in_maps(x, combine_array, dispatch_mask, weight, bias)
    res = run_spmd(in_maps)
    out = np.empty((B, T, OUT), dtype=np.float32)
    for k in range(8):
        b, oh = k // 2, k % 2
        out[b, :, oh * OH : (oh + 1) * OH] = res.results[k]["out"]
    return out


# revision 9
# speedup vs baseline: 1.3847x; 1.0443x over previous
"""Trainium2 Bass kernel for ExpertsChooseMaskedExpand MoE routing.

Math (reference):
    xd[b,e,c,i] = sum_t x[b,t,(e,i)] * dmask[b,t,e,c]            (dispatch)
    y[b,e,c,o]  = sum_i xd[b,e,c,i] * w[e,o,i] + bias[o]         (expert mm)
    out[b,t,o]  = sum_{e,c} y[b,e,c,o] * cmb[b,t,e,c]            (combine)

Restructured (combine applied before the weight matmul — 155 GF total
instead of 215 GF; the E expert matmuls fuse into one K=2048 matmul):
    xd[b,e][c,j] = sum_t dmask[b,e][t,c] * xr[b,e][t,j]
    zT[b,e][j,t] = sum_c xd[b,e][c,j] * cmbT[b,e][c,t]
    out[b][t,o]  = sum_{(e,j)} zT[b][(e,j),t] * wstack[(e,j),o] + s[b][t]*bias[o]
    where s[b][t] = sum_{e,c} cmb[b,t,e,c],  wstack[(e,j),o] = w[e,o,j]

Sharding: 8 cores = (batch b in 0..3) x (output half oh in 0..1). Each
core computes its exact out[b][:, oh*4096:(oh+1)*4096] slice - no
cross-core reduction. All matmuls run as float32r (fp22, full PE rate).
The s[t]*bias[o] rank-1 term is a K=1 matmul appended to each
accumulation group.
"""

import numpy as np

B, T, E, C = 4, 1024, 4, 512
IN, OUT = 2048, 8192
P = 128
TT = T // P          # 8  t-tiles
CT = C // P          # 4  c-tiles per expert
JT = 4               # j-tiles per expert (i = 512)
KT = E * JT          # 16 k-tiles for the fused matmul (K = 2048)
OH = OUT // 2        # 4096 output columns per core
OC = OH // 512       # 8  output chunks of 512
WQ = 4               # w quarters per oc (4 kt each)

_CACHE = {}


def _build_nc():
    import concourse.mybir as mybir
    import concourse.tile as tile
    from concourse import bacc

    f32 = mybir.dt.float32
    f32r = mybir.dt.float32r

    nc = bacc.Bacc("TRN2", target_bir_lowering=False, debug=False, num_devices=8)
    x_t = nc.dram_tensor("x", (T, IN), f32r, kind="ExternalInput")
    dm_t = nc.dram_tensor("dm", (T, E, C), f32r, kind="ExternalInput")
    cT_t = nc.dram_tensor("cmbT", (E, C, T), f32r, kind="ExternalInput")
    wT_t = nc.dram_tensor("wT", (KT, P, OH), f32r, kind="ExternalInput")
    sT_t = nc.dram_tensor("sT", (P, TT), f32, kind="ExternalInput")
    bb_t = nc.dram_tensor("biasb", (P, OH), f32, kind="ExternalInput")
    o_t = nc.dram_tensor("out", (T, OH), f32, kind="ExternalOutput")

    x_r = x_t.ap().rearrange("(tt p) f -> p tt f", p=P)        # [128, 8, 2048]
    dm_r = dm_t.ap().rearrange("(tt p) e c -> p tt e c", p=P)  # [128, 8, 4, 512]
    cT_r = cT_t.ap().rearrange("e (ct p) t -> p e ct t", p=P)  # [128, 4, 4, 1024]
    wT_r = wT_t.ap().rearrange("kt p o -> p kt o")             # [128, 16, 4096]
    o_r = o_t.ap().rearrange("(tt p) o -> p tt o", p=P)        # [128, 8, 4096]

    with tile.TileContext(nc) as tc:
        with (
            tc.tile_pool(name="persist", bufs=1) as persist,
            tc.tile_pool(name="wp", bufs=8) as wp,
            tc.tile_pool(name="xdm", bufs=4) as xdm,
            tc.tile_pool(name="cp", bufs=3) as cp,
            tc.tile_pool(name="xdp", bufs=1) as xdp,
            tc.tile_pool(name="op", bufs=2) as op,
        ):
            zT = persist.tile([P, KT, T], f32r)       # 64 KiB/partition
            sT_sb = persist.tile([P, TT], f32)
            nc.sync.dma_start(sT_sb, sT_t.ap())

            w_tiles = {}

            def load_w(oc, q):
                t = wp.tile([P, WQ, 512], f32r, tag="w", name=f"w_{oc}_{q}")
                nc.sync.dma_start(
                    t, wT_r[:, WQ * q : WQ * (q + 1), oc * 512 : (oc + 1) * 512]
                )
                w_tiles[(oc, q)] = t

            # ---- Phases 1+2: per-expert dispatch and combine ----
            with (
                tc.tile_pool(name="ps_a", bufs=4, space="PSUM") as ps_a,
                tc.tile_pool(name="ps_b", bufs=2, space="PSUM") as ps_b,
            ):
                for e in range(E):
                    # phase 1: xd[c, j] = sum_t dm[t, c] * x[t, j]
                    # quarter-granularity loads; tt-outer so matmuls start early
                    xq, dmq = {}, {}
                    for qt in range(4):
                        qs = slice(qt * 2, qt * 2 + 2)
                        xq[qt] = xdm.tile(
                            [P, 2, 512], f32r, tag="x", name=f"x_{e}_{qt}"
                        )
                        dmq[qt] = xdm.tile(
                            [P, 2, 512], f32r, tag="dm", name=f"dm_{e}_{qt}"
                        )
                        nc.sync.dma_start(
                            xq[qt], x_r[:, qs, e * 512 : (e + 1) * 512]
                        )
                        nc.sync.dma_start(dmq[qt], dm_r[:, qs, e, :])
                    # combine loads for this expert, issued before phase-1 runs
                    c_ths = []
                    for th in range(2):
                        c_th = cp.tile([P, CT, 512], f32r, tag="c", name=f"c_{e}_{th}")
                        nc.sync.dma_start(
                            c_th, cT_r[:, e, :, th * 512 : (th + 1) * 512]
                        )
                        c_ths.append(c_th)
                    ps1 = [
                        ps_a.tile([P, 512], f32, tag="ps1", name=f"ps1_{e}_{ct}")
                        for ct in range(CT)
                    ]
                    for tt in range(TT):
                        qt, qi = tt // 2, tt % 2
                        for ct in range(CT):
                            nc.tensor.matmul(
                                ps1[ct],
                                dmq[qt][:, qi, ct * P : (ct + 1) * P],
                                xq[qt][:, qi, :],
                                start=(tt == 0),
                                stop=(tt == TT - 1),
                            )
                    xd_e = xdp.tile([P, CT, 512], f32r, tag="xd")
                    for ct in range(CT):
                        nc.vector.tensor_copy(xd_e[:, ct, :], ps1[ct])

                    # phase 2: zT[j, t] = sum_c xd[c, j] * cmbT[c, t]
                    for th in range(2):
                        c_th = c_ths[th]
                        for jt in range(JT):
                            ps2 = ps_b.tile([P, 512], f32, tag="ps2")
                            for ct in range(CT):
                                nc.tensor.matmul(
                                    ps2,
                                    xd_e[:, ct, jt * P : (jt + 1) * P],
                                    c_th[:, ct, :],
                                    start=(ct == 0),
                                    stop=(ct == CT - 1),
                                )
                            nc.vector.tensor_copy(
                                zT[:, e * JT + jt, th * 512 : (th + 1) * 512], ps2
                            )

                    # weight prefetch at the tail of each expert's DMA stream
                    tail_w = {
                        2: [(0, 0), (0, 1)],
                        3: [(0, 2), (0, 3), (1, 0), (1, 1)],
                    }.get(e, [])
                    for oc, q in tail_w:
                        load_w(oc, q)

            # ---- Phase 3: out[t,o] = sum_kt zT[kt].T @ w[kt] + s^T bias ----
            with tc.tile_pool(name="ps_c", bufs=4, space="PSUM") as ps_c:
                for oc in range(OC):
                    for q in range(WQ):
                        if (oc, q) not in w_tiles:
                            load_w(oc, q)
                    bias_oc = op.tile([P, 512], f32, tag="bias_oc", bufs=1)
                    nc.sync.dma_start(
                        bias_oc, bb_t.ap()[:, oc * 512 : (oc + 1) * 512]
                    )
                    for tt in range(TT):
                        # spread weight prefetch for oc+1 across this oc
                        if tt in (1, 3, 5, 7) and oc + 1 < OC:
                            pq = (tt - 1) // 2
                            if (oc + 1, pq) not in w_tiles:
                                load_w(oc + 1, pq)
                        ps3 = ps_c.tile([P, 512], f32, tag="ps3")
                        for kt in range(KT):
                            nc.tensor.matmul(
                                ps3,
                                zT[:, kt, tt * P : (tt + 1) * P],
                                w_tiles[(oc, kt // WQ)][:, kt % WQ, :],
                                start=(kt == 0),
                                stop=(kt == KT - 1),
                            )
                        o_sb = op.tile([P, 512], f32, tag="o_sb")
                        # out = biasb[:, oc] * sT[:, tt] + psum
                        nc.vector.scalar_tensor_tensor(
                            o_sb,
                            bias_oc,
                            sT_sb[:, tt : tt + 1],
                            ps3,
                            mybir.AluOpType.mult,
                            mybir.AluOpType.add,
                        )
                        nc.sync.dma_start(
                            o_r[:, tt, oc * 512 : (oc + 1) * 512], o_sb
                        )

    nc.compile()
    return nc


def _get_nc():
    if "nc" not in _CACHE:
        _CACHE["nc"] = _build_nc()
    return _CACHE["nc"]


def _prep_in_maps(x, combine_array, dispatch_mask, weight, bias):
    x = np.ascontiguousarray(x, dtype=np.float32)
    cmb = np.ascontiguousarray(combine_array, dtype=np.float32)
    dm = np.ascontiguousarray(dispatch_mask, dtype=np.float32)
    weight = np.ascontiguousarray(weight, dtype=np.float32)
    bias = np.ascontiguousarray(bias, dtype=np.float32)

    # combine transposed to (B, E, C, T) so that C lands on partitions
    cmbT = np.ascontiguousarray(cmb.transpose(0, 2, 3, 1))
    s = cmb.sum(axis=(2, 3))  # (B, T)
    sT = [np.ascontiguousarray(s[b].reshape(TT, P).T) for b in range(B)]  # (P, TT)
    # wstack[(e,j), o] = w[e, o, j];  w = weight.reshape(E, OUT, IN//E)
    w = weight.reshape(E, OUT, IN // E)
    wstack = np.ascontiguousarray(w.transpose(0, 2, 1)).reshape(IN, OUT)
    wT = [
        np.ascontiguousarray(wstack[:, oh * OH : (oh + 1) * OH]).reshape(KT, P, OH)
        for oh in range(2)
    ]
    biasb = [
        np.ascontiguousarray(np.broadcast_to(bias[oh * OH : (oh + 1) * OH], (P, OH)))
        for oh in range(2)
    ]

    in_maps = []
    for k in range(8):
        b, oh = k // 2, k % 2
        in_maps.append(
            {
                "x": x[b],
                "dm": dm[b],
                "cmbT": cmbT[b],
                "wT": wT[oh],
                "sT": sT[b],
                "biasb": biasb[oh],
            }
        )
    return in_maps


def run_spmd(in_maps, trace=False, **kwargs):
    from concourse.bass_utils import run_bass_kernel_spmd

    nc = _get_nc()
    return run_bass_kernel_spmd(
        nc, in_maps, core_ids=list(range(8)), trace=trace, **kwargs
    )


def kernel(x, combine_array, dispatch_mask, weight, bias, num_experts):
    assert int(num_experts) == E
    in_maps = _prep_in_maps(x, combine_array, dispatch_mask, weight, bias)
    res = run_spmd(in_maps)
    out = np.empty((B, T, OUT), dtype=np.float32)
    for k in range(8):
        b, oh = k // 2, k % 2
        out[b, :, oh * OH : (oh + 1) * OH] = res.results[k]["out"]
    return out
